# revision 1
# baseline (speedup 1.0000x reference)
"""Multi-head latent attention (MLA) TRN2 kernel.

Sharding: batch(2) x query-sequence(4) over 8 cores. Each core:
  - computes the full KV path for its batch (kv_a, rmsnorm, kv_b, rope)
  - computes the Q path for its 512-token query chunk
  - full attention for its 512 queries x 2048 keys x 16 heads
  - o_proj for its chunk -> output slice [512, 2048]
Host assembles the 8 slices into [B, T, HID]. No collectives.

All matmuls run in float32r (fp32 with 11-bit mantissa, 1 cycle/row on the
PE when N>=256 -- same throughput as bf16 at ~2^-12 relative precision).
Activations are kept feature-major ([feature, token]) so weight matrices act
as lhsT directly as stored; attention computes scores transposed
(s^T[k,q] = k^T q) so softmax needs no transposes: exp on ACT, the
denominator via an all-ones lhsT matmul (broadcast into all 128 partitions),
and P@V consumes the transposed probabilities directly.
"""

import math

import numpy as np

B, T, HID = 2, 2048, 2048
NH, NKV = 16, 8
NOPE, ROPE = 128, 64
HD = NOPE + ROPE  # 192
VD = 128
KV_RANK, Q_RANK = 512, 1536
EPS = 1e-6
THETA = 10000.0
NCORES = 8
TQ = B * T // NCORES  # 512 query tokens per core
P = 128
SCALE = 1.0 / math.sqrt(HD)

# Rope rows are stored "paired": each head's rotated rope halves (32+32 rows)
# are stacked into one contiguous 64-row slot, two heads per 128-partition
# tile, at base partition 64*(kvh%2) so score-matmul lhsT(k)/rhs(q) base
# partitions match (PE only allows bases {0, 32, 64}).

_CACHE = {}


def _round_f32r(a):
    a = np.ascontiguousarray(np.asarray(a, dtype=np.float32))
    u = a.view(np.uint32)
    low = u & np.uint32(0xFFF)
    rounded = u & np.uint32(0xFFFFF000)
    lsb = (u >> np.uint32(12)) & np.uint32(1)
    round_up = (low > 0x800) | ((low == 0x800) & (lsb == 1))
    return (rounded + (round_up.astype(np.uint32) << np.uint32(12))).view(np.float32)


def _build_nc():
    import concourse.bass as bass  # noqa: F401
    import concourse.mybir as mybir
    from concourse import bacc
    from concourse.tile import TileContext

    F32 = mybir.dt.float32
    F32R = mybir.dt.float32r
    AF = mybir.ActivationFunctionType
    ALU = mybir.AluOpType

    nc = bacc.Bacc(None, target_bir_lowering=False)

    xT = nc.dram_tensor("xT", [HID, T], F32R, kind="ExternalInput")
    xq = nc.dram_tensor("xq", [HID, TQ], F32R, kind="ExternalInput")
    qa_w = nc.dram_tensor("qa_w", [HID, Q_RANK], F32R, kind="ExternalInput")
    qa_ln = nc.dram_tensor("qa_ln", [P, Q_RANK // P], F32R, kind="ExternalInput")
    qb_w = nc.dram_tensor("qb_w", [Q_RANK, NH * HD], F32R, kind="ExternalInput")
    kva_w = nc.dram_tensor("kva_w", [HID, KV_RANK + NKV * ROPE], F32R, kind="ExternalInput")
    kva_ln = nc.dram_tensor("kva_ln", [P, KV_RANK // P], F32R, kind="ExternalInput")
    kvb_w = nc.dram_tensor("kvb_w", [KV_RANK, NKV * (NOPE + VD)], F32R, kind="ExternalInput")
    o_w = nc.dram_tensor("o_w", [NH * VD, HID], F32R, kind="ExternalInput")
    cosq = nc.dram_tensor("cosq", [P, TQ], F32R, kind="ExternalInput")
    sinq = nc.dram_tensor("sinq", [P, TQ], F32R, kind="ExternalInput")
    cosk = nc.dram_tensor("cosk", [P, T], F32R, kind="ExternalInput")
    sink = nc.dram_tensor("sink", [P, T], F32R, kind="ExternalInput")
    ones_in = nc.dram_tensor("ones_in", [P, P], F32R, kind="ExternalInput")
    eps_in = nc.dram_tensor("eps_in", [P, 2], F32, kind="ExternalInput")
    out = nc.dram_tensor("out", [TQ, HID], F32, kind="ExternalOutput")

    xT_t = xT.rearrange("(kt p) t -> p kt t", p=P)  # [128, 16, T]
    xq_t = xq.rearrange("(kt p) t -> p kt t", p=P)  # [128, 16, TQ]

    with TileContext(nc) as tc:
        with (
            tc.tile_pool(name="tables", bufs=1) as tbl,
            tc.tile_pool(name="dram", bufs=1, space="DRAM") as dpool,
            tc.tile_pool(name="pAttn", bufs=1) as pAttn,
        ):
            ones_sb = tbl.tile([P, P], F32R, name="ones_sb")
            nc.sync.dma_start(ones_sb[:], ones_in[:, :])
            lnq_sb = tbl.tile([P, Q_RANK // P], F32R, name="lnq_sb")
            nc.sync.dma_start(lnq_sb[:], qa_ln[:, :])
            lnkv_sb = tbl.tile([P, KV_RANK // P], F32R, name="lnkv_sb")
            nc.sync.dma_start(lnkv_sb[:], kva_ln[:, :])
            eps_sb = tbl.tile([P, 2], F32, name="eps_sb")
            nc.sync.dma_start(eps_sb[:], eps_in[:, :])
            epskv_sb = eps_sb[:, 0:1]
            epsq_sb = eps_sb[:, 1:2]

            kpaird = dpool.tile([P, 4, T], F32R, name="kpaird")
            qnoped = dpool.tile([P, NH, TQ], F32R, name="qnoped")
            qpaird = dpool.tile([P, 8, TQ], F32R, name="qpaird")

            # attention output, resident through P3+P4
            attn_sb = pAttn.tile([P, NH, TQ], F32R, name="attn_sb")

            with tc.tile_pool(name="pLat", bufs=1) as pLat:
                kv_latN = pLat.tile([P, 4, T], F32R, name="kv_latN")

                # ------------- P2: q path (first; no kv deps) ---------------
                with (
                    tc.tile_pool(name="p2", bufs=1) as p2,
                    tc.tile_pool(name="p2s", bufs=2) as p2s,
                    tc.tile_pool(name="p2w", bufs=3) as p2w,
                    tc.tile_pool(name="p2ps", bufs=2, space="PSUM") as p2ps,
                    tc.tile_pool(name="p2ps1", bufs=1, space="PSUM") as p2ps1,
                ):
                    q_lat = p2.tile([P, Q_RANK // P, TQ], F32R, name="q_lat")
                    rs_q = p2.tile([P, TQ], F32, name="rs_q")

                    with tc.tile_pool(name="p2xq", bufs=1) as p2xq:
                        xq_c = []
                        for c in range(4):
                            t_ = p2xq.tile([P, 4, TQ], F32R, name=f"xq_c{c}")
                            nc.sync.dma_start(t_[:], xq_t[:, 4 * c : 4 * c + 4, :])
                            xq_c.append(t_)

                        # q_a + rmsnorm
                        sumsq = p2ps1.tile([P, TQ], F32, tag="qsumsq")
                        for m in range(12):
                            wt = p2w.tile([P, 16, P], F32R, tag="qa_wt")
                            nc.sync.dma_start(
                                wt[:],
                                qa_w.rearrange("(kt p) c -> p kt c", p=P)[
                                    :, :, m * P : (m + 1) * P
                                ],
                            )
                            ps = p2ps.tile([P, TQ], F32, tag="qa_ps")
                            for k in range(16):
                                nc.tensor.matmul(
                                    ps[:], wt[:, k, :], xq_c[k // 4][:, k % 4, :],
                                    start=(k == 0), stop=(k == 15),
                                )
                            nc.vector.tensor_copy(q_lat[:, m, :], ps[:])
                            sq = p2s.tile([P, TQ], F32R, tag="qsq")
                            nc.scalar.square(sq[:], ps[:])
                            nc.tensor.matmul(
                                sumsq[:], ones_sb[:], sq[:],
                                start=(m == 0), stop=(m == 11),
                            )
                        sqt = p2s.tile([P, TQ], F32, tag="qsqt")
                        nc.scalar.activation(sqt[:], sumsq[:], AF.Sqrt, bias=epsq_sb[:])
                        nc.vector.reciprocal(rs_q[:], sqt[:])
                        for m in range(Q_RANK // P):
                            nc.vector.scalar_tensor_tensor(
                                q_lat[:, m, :], q_lat[:, m, :],
                                lnq_sb[:, m : m + 1], rs_q[:],
                                ALU.mult, ALU.mult,
                            )

                    # q_b: nope tiles spill to HBM; rope raw kept for rotation
                    with tc.tile_pool(name="p2b", bufs=1) as p2b:
                        qraw1 = p2b.tile([P, 4, TQ], F32R, name="qraw1")
                        qraw2 = p2b.tile([P, 4, TQ], F32R, name="qraw2")
                        for m in range(24):
                            wt = p2w.tile([P, 12, P], F32R, tag="qb_wt")
                            nc.sync.dma_start(
                                wt[:],
                                qb_w.rearrange("(kt p) c -> p kt c", p=P)[
                                    :, :, m * P : (m + 1) * P
                                ],
                            )
                            ps = p2ps.tile([P, TQ], F32, tag="qb_ps")
                            for k in range(12):
                                nc.tensor.matmul(
                                    ps[:], wt[:, k, :], q_lat[:, k, :],
                                    start=(k == 0), stop=(k == 11),
                                )
                            if m < 16:
                                st = p2s.tile([P, TQ], F32R, tag="qn_st")
                                nc.scalar.copy(st[:], ps[:])
                                nc.sync.dma_start(qnoped[:, m, :], st[:])
                            elif m < 20:
                                nc.scalar.copy(qraw1[:, m - 16, :], ps[:])
                            else:
                                nc.scalar.copy(qraw2[:, m - 20, :], ps[:])

                        # q-rope rotation then scatter to paired HBM layout
                        cosq_sb = p2b.tile([P, TQ], F32R, name="cosq_sb")
                        nc.sync.dma_start(cosq_sb[:], cosq[:, :])
                        sinq_sb = p2b.tile([P, TQ], F32R, name="sinq_sb")
                        nc.sync.dma_start(sinq_sb[:], sinq[:, :])
                        cb = cosq_sb[:, None, :].to_broadcast((P, 4, TQ))
                        sb = sinq_sb[:, None, :].to_broadcast((P, 4, TQ))
                        qrot1 = p2b.tile([P, 4, TQ], F32R, name="qrot1")
                        qrot2 = p2b.tile([P, 4, TQ], F32R, name="qrot2")
                        tmp = p2b.tile([P, 4, TQ], F32R, name="qrot_tmp1")
                        nc.vector.tensor_tensor(tmp[:], qraw2[:], sb, ALU.mult)
                        nc.vector.tensor_tensor(qrot1[:], qraw1[:], cb, ALU.mult)
                        nc.vector.tensor_tensor(qrot1[:], qrot1[:], tmp[:], ALU.subtract)
                        tmp2 = p2b.tile([P, 4, TQ], F32R, name="qrot_tmp2")
                        nc.vector.tensor_tensor(tmp2[:], qraw1[:], sb, ALU.mult)
                        nc.vector.tensor_tensor(qrot2[:], qraw2[:], cb, ALU.mult)
                        nc.vector.tensor_tensor(qrot2[:], qrot2[:], tmp2[:], ALU.add)
                        # head h -> tile 2*(h//4)+h%2, base 64*((h//2)%2)
                        for h in range(NH):
                            tq_ = 2 * (h // 4) + h % 2
                            bb = 64 * ((h // 2) % 2)
                            nc.sync.dma_start(
                                qpaird[bb : bb + 32, tq_, :],
                                qrot1[(h % 4) * 32 : (h % 4) * 32 + 32, h // 4, :],
                            )
                            nc.sync.dma_start(
                                qpaird[bb + 32 : bb + 64, tq_, :],
                                qrot2[(h % 4) * 32 : (h % 4) * 32 + 32, h // 4, :],
                            )

                # ------------- P1: kv_a + rmsnorm + interleaved rotation ----
                with (
                    tc.tile_pool(name="p1", bufs=1) as p1,
                    tc.tile_pool(name="p1s", bufs=2) as p1s,
                    tc.tile_pool(name="p1ps", bufs=2, space="PSUM") as p1ps,
                    tc.tile_pool(name="p1ps1", bufs=1, space="PSUM") as p1ps1,
                ):
                    kvaw_c = []
                    for c in range(4):
                        t_ = p1.tile([P, 16, 256], F32R, name=f"kvaw_c{c}")
                        nc.sync.dma_start(
                            t_[:],
                            kva_w.rearrange("(kt p) c -> p kt c", p=P)[
                                :, :, c * 256 : (c + 1) * 256
                            ],
                        )
                        kvaw_c.append(t_)

                    def kvaw_at(k, m):
                        return kvaw_c[m // 2][:, k, (m % 2) * P : (m % 2 + 1) * P]

                    cosk_sb = p1.tile([P, T], F32R, name="cosk_sb")
                    nc.sync.dma_start(cosk_sb[:], cosk[:, :])
                    sink_sb = p1.tile([P, T], F32R, name="sink_sb")
                    nc.sync.dma_start(sink_sb[:], sink[:, :])
                    rs_kv = p1.tile([P, 8, 256], F32, name="rs_kv")

                    NCH = 8
                    CW = T // NCH  # 256
                    for nch in range(NCH):
                        chsl = slice(nch * CW, (nch + 1) * CW)
                        xch = p1s.tile([P, 16, CW], F32R, tag="xch")
                        nc.sync.dma_start(xch[:], xT_t[:, :, chsl])
                        sumsq = p1ps1.tile([P, CW], F32, tag="sumsq")
                        raw1 = p1s.tile([P, 2, CW], F32R, tag="kraw1")
                        raw2 = p1s.tile([P, 2, CW], F32R, tag="kraw2")
                        for m in range(8):
                            ps = p1ps.tile([P, CW], F32, tag="kva_ps")
                            for k in range(16):
                                nc.tensor.matmul(
                                    ps[:], kvaw_at(k, m), xch[:, k, :],
                                    start=(k == 0), stop=(k == 15),
                                )
                            if m < 4:
                                nc.vector.tensor_copy(kv_latN[:, m, chsl], ps[:])
                                sq = p1s.tile([P, CW], F32R, tag="sq")
                                nc.scalar.square(sq[:], ps[:])
                                nc.tensor.matmul(
                                    sumsq[:], ones_sb[:], sq[:],
                                    start=(m == 0), stop=(m == 3),
                                )
                            elif m < 6:
                                nc.scalar.copy(raw1[:, m - 4, :], ps[:])
                            else:
                                nc.scalar.copy(raw2[:, m - 6, :], ps[:])
                        sqt = p1s.tile([P, CW], F32, tag="sqt")
                        nc.scalar.activation(sqt[:], sumsq[:], AF.Sqrt, bias=epskv_sb[:])
                        nc.vector.reciprocal(rs_kv[:, nch, :], sqt[:])
                        for m in range(4):
                            nc.vector.scalar_tensor_tensor(
                                kv_latN[:, m, chsl],
                                kv_latN[:, m, chsl],
                                lnkv_sb[:, m : m + 1],
                                rs_kv[:, nch, :],
                                ALU.mult,
                                ALU.mult,
                            )
                        # rotate this chunk's rope rows and scatter to HBM
                        for t in range(2):
                            tmp = p1s.tile([P, CW], F32R, tag="rot_tmp")
                            rot = p1s.tile([P, CW], F32R, tag="rot_out")
                            nc.vector.tensor_tensor(
                                tmp[:], raw2[:, t, :], sink_sb[:, chsl], ALU.mult
                            )
                            nc.vector.tensor_tensor(
                                rot[:], raw1[:, t, :], cosk_sb[:, chsl], ALU.mult
                            )
                            nc.vector.tensor_tensor(rot[:], rot[:], tmp[:], ALU.subtract)
                            tmp2 = p1s.tile([P, CW], F32R, tag="rot_tmp")
                            rot2 = p1s.tile([P, CW], F32R, tag="rot_out")
                            nc.vector.tensor_tensor(
                                tmp2[:], raw1[:, t, :], sink_sb[:, chsl], ALU.mult
                            )
                            nc.vector.tensor_tensor(
                                rot2[:], raw2[:, t, :], cosk_sb[:, chsl], ALU.mult
                            )
                            nc.vector.tensor_tensor(rot2[:], rot2[:], tmp2[:], ALU.add)
                            # head kvh=4t+i -> tile kvh//2, base 64*(kvh%2)
                            for i in range(4):
                                kvh = 4 * t + i
                                bb = 64 * (kvh % 2)
                                nc.sync.dma_start(
                                    kpaird[bb : bb + 32, kvh // 2, chsl],
                                    rot[i * 32 : (i + 1) * 32, :],
                                )
                                nc.sync.dma_start(
                                    kpaird[bb + 32 : bb + 64, kvh // 2, chsl],
                                    rot2[i * 32 : (i + 1) * 32, :],
                                )

                # ------------- P3: attention --------------------------------
                with (
                    tc.tile_pool(name="p3s", bufs=2) as p3s,
                    tc.tile_pool(name="p3q", bufs=4) as p3q,
                    tc.tile_pool(name="p3p", bufs=3) as p3p,
                    tc.tile_pool(name="scps", bufs=3, space="PSUM") as scps,
                    tc.tile_pool(name="atps", bufs=2, space="PSUM") as atps,
                    tc.tile_pool(name="prps", bufs=2, space="PSUM") as prps,
                ):
                    pending = []

                    def finalize(item):
                        dsum, at, qh = item
                        dn = scps.tile([P, TQ], F32, tag="sc")
                        nc.tensor.matmul(
                            dn[:], ones_sb[:], dsum[:], start=True, stop=True
                        )
                        rec = p3q.tile([P, TQ], F32, tag="rec")
                        nc.vector.reciprocal(rec[:], dn[:])
                        nc.vector.tensor_tensor(
                            attn_sb[:, qh, :], at[:], rec[:], ALU.mult
                        )

                    for hp in range(4):  # kv-head pairs
                        kvh0 = 2 * hp
                        wn = p3s.tile([P, 4, 256], F32R, tag="wn")
                        nc.sync.dma_start(
                            wn[:],
                            kvb_w.rearrange("(kt p) c -> p kt c", p=P)[
                                :, :, kvh0 * NOPE : (kvh0 + 2) * NOPE
                            ],
                        )
                        wv = p3s.tile([P, 4, 256], F32R, tag="wv")
                        nc.sync.dma_start(
                            wv[:],
                            kvb_w.rearrange("(kt p) c -> p kt c", p=P)[
                                :, :, NKV * NOPE + kvh0 * VD : NKV * NOPE + (kvh0 + 2) * VD
                            ],
                        )
                        knp = p3s.tile([P, 2, T], F32R, tag="knp")
                        for h2 in range(2):
                            for nch in range(4):
                                ps = prps.tile([P, 512], F32, tag="pr_ps")
                                for k in range(4):
                                    nc.tensor.matmul(
                                        ps[:],
                                        wn[:, k, h2 * P : (h2 + 1) * P],
                                        kv_latN[:, k, nch * 512 : (nch + 1) * 512],
                                        start=(k == 0),
                                        stop=(k == 3),
                                    )
                                nc.vector.tensor_copy(
                                    knp[:, h2, nch * 512 : (nch + 1) * 512], ps[:]
                                )
                        vp = p3s.tile([P, 16, 256], F32R, tag="vp")
                        for mt in range(16):
                            psf = prps.tile([P, 512], F32, tag="pr_ps")
                            ps = psf[:, :256]
                            for k in range(4):
                                nc.tensor.matmul(
                                    ps[:],
                                    kv_latN[:, k, mt * P : (mt + 1) * P],
                                    wv[:, k, :],
                                    start=(k == 0),
                                    stop=(k == 3),
                                )
                            nc.vector.tensor_copy(vp[:, mt, :], ps[:])
                        krp = p3s.tile([P, T], F32R, tag="krp")
                        nc.sync.dma_start(krp[:], kpaird[:, hp, :])
                        qps = {}
                        for tq_ in (2 * hp, 2 * hp + 1):
                            qp = p3q.tile([P, TQ], F32R, tag="qp")
                            nc.sync.dma_start(qp[:], qpaird[:, tq_, :])
                            qps[tq_] = qp

                        for j4 in range(4):
                            qh = 4 * hp + j4
                            kvh = qh // 2
                            h2 = kvh - kvh0
                            b = 64 * (kvh % 2)
                            tq_ = 2 * (qh // 4) + qh % 2
                            qn = p3q.tile([P, TQ], F32R, tag="qn")
                            nc.sync.dma_start(qn[:], qnoped[:, qh, :])
                            qp = qps[tq_]
                            dsum = p3q.tile([P, TQ], F32R, tag="dsum")
                            at = atps.tile([P, TQ], F32, tag="at")
                            pts = {}
                            for kt in range(16):
                                sc = scps.tile([P, TQ], F32, tag="sc")
                                nc.tensor.matmul(
                                    sc[:],
                                    knp[:, h2, kt * P : (kt + 1) * P],
                                    qn[:],
                                    start=True,
                                    stop=False,
                                )
                                nc.tensor.matmul(
                                    sc[:],
                                    krp[b : b + 64, kt * P : (kt + 1) * P],
                                    qp[b : b + 64, :],
                                    start=False,
                                    stop=True,
                                )
                                pt = p3p.tile([P, TQ], F32R, tag="probsT")
                                nc.scalar.activation(
                                    pt[:], sc[:], AF.Exp, scale=float(SCALE)
                                )
                                pts[kt] = pt
                                if kt == 0:
                                    nc.vector.tensor_copy(dsum[:], pt[:])
                                else:
                                    nc.vector.tensor_tensor(
                                        dsum[:], dsum[:], pt[:], ALU.add
                                    )
                                if kt > 0:  # PV one stage behind scores
                                    nc.tensor.matmul(
                                        at[:],
                                        vp[:, kt - 1, h2 * P : (h2 + 1) * P],
                                        pts[kt - 1][:],
                                        start=(kt == 1),
                                        stop=False,
                                    )
                                    del pts[kt - 1]
                            nc.tensor.matmul(
                                at[:],
                                vp[:, 15, h2 * P : (h2 + 1) * P],
                                pts[15][:],
                                start=False,
                                stop=True,
                            )
                            pending.append((dsum, at, qh))
                            if len(pending) == 2:
                                finalize(pending.pop(0))
                    while pending:
                        finalize(pending.pop(0))

            # ------------- P4: o_proj (attn_sb resident) --------------------
            with (
                tc.tile_pool(name="p4s", bufs=2) as p4s,
                tc.tile_pool(name="p4ps", bufs=2, space="PSUM") as p4ps,
            ):
                for n in range(4):
                    ow = p4s.tile([P, 16, 512], F32R, tag="ow")
                    nc.sync.dma_start(
                        ow[:],
                        o_w.rearrange("(ht p) c -> p ht c", p=P)[
                            :, :, n * 512 : (n + 1) * 512
                        ],
                    )
                    for mt in range(4):
                        ps = p4ps.tile([P, 512], F32, tag="o_ps")
                        for h in range(NH):
                            nc.tensor.matmul(
                                ps[:],
                                attn_sb[:, h, mt * P : (mt + 1) * P],
                                ow[:, h, :],
                                start=(h == 0),
                                stop=(h == 15),
                            )
                        st = p4s.tile([P, 512], mybir.dt.float32, tag="ost")
                        nc.scalar.copy(st[:], ps[:])
                        nc.sync.dma_start(
                            out[mt * P : (mt + 1) * P, n * 512 : (n + 1) * 512], st[:]
                        )

    nc.finalize()
    return nc


def _host_prep(inputs):
    r = _round_f32r
    x = np.asarray(inputs["hidden_states"], dtype=np.float32)
    qa_w = r(inputs["q_a_w"])
    qa_ln = r(
        (np.asarray(inputs["q_a_ln_w"], np.float64) * math.sqrt(Q_RANK))
        .astype(np.float32)
        .reshape(Q_RANK // P, P)
        .T.copy()
    )
    kva_ln = r(
        (np.asarray(inputs["kv_a_ln_w"], np.float64) * math.sqrt(KV_RANK))
        .astype(np.float32)
        .reshape(KV_RANK // P, P)
        .T.copy()
    )
    o_w = r(inputs["o_w"])

    qb = np.asarray(inputs["q_b_w"], np.float32).reshape(Q_RANK, NH, HD)
    nope_cols = qb[:, :, :NOPE].reshape(Q_RANK, NH * NOPE)
    rope1 = qb[:, :, NOPE : NOPE + 32].reshape(Q_RANK, 16 * 32)
    rope2 = qb[:, :, NOPE + 32 :].reshape(Q_RANK, 16 * 32)
    qb_w = r(np.concatenate([nope_cols, rope1, rope2], axis=1))

    kva = np.asarray(inputs["kv_a_w"], np.float32)
    lat = kva[:, :KV_RANK]
    krope = kva[:, KV_RANK:].reshape(HID, NKV, ROPE)
    kr1 = krope[:, :, :32].reshape(HID, NKV * 32)
    kr2 = krope[:, :, 32:].reshape(HID, NKV * 32)
    kva_w = r(np.concatenate([lat, kr1, kr2], axis=1))

    kvb = np.asarray(inputs["kv_b_w"], np.float32).reshape(KV_RANK, NKV, NOPE + VD)
    knope_cols = kvb[:, :, :NOPE].reshape(KV_RANK, NKV * NOPE)
    v_cols = kvb[:, :, NOPE:].reshape(KV_RANK, NKV * VD)
    kvb_w = r(np.concatenate([knope_cols, v_cols], axis=1))

    inv_freq = 1.0 / (THETA ** (np.arange(0, ROPE, 2, dtype=np.float32) / ROPE))
    t = np.arange(T, dtype=np.float32)
    freqs = np.outer(t, inv_freq).astype(np.float32)
    cosk = r(np.tile(np.cos(freqs).T, (4, 1)))  # [128, T]
    sink = r(np.tile(np.sin(freqs).T, (4, 1)))
    ones = np.ones((P, P), np.float32)
    eps2 = np.empty((P, 2), np.float32)
    eps2[:, 0] = EPS * KV_RANK
    eps2[:, 1] = EPS * Q_RANK

    in_maps = []
    for c in range(NCORES):
        b, qc = c // 4, c % 4
        xTb = r(x[b].T.copy())
        qoff = qc * TQ
        in_maps.append(
            {
                "xT": xTb,
                "xq": np.ascontiguousarray(xTb[:, qoff : qoff + TQ]),
                "qa_w": qa_w,
                "qa_ln": qa_ln,
                "qb_w": qb_w,
                "kva_w": kva_w,
                "kva_ln": kva_ln,
                "kvb_w": kvb_w,
                "o_w": o_w,
                "cosq": np.ascontiguousarray(cosk[:, qoff : qoff + TQ]),
                "sinq": np.ascontiguousarray(sink[:, qoff : qoff + TQ]),
                "cosk": cosk,
                "sink": sink,
                "ones_in": ones,
                "eps_in": eps2,
            }
        )
    return in_maps


def get_nc():
    if "nc" not in _CACHE:
        _CACHE["nc"] = _build_nc()
    return _CACHE["nc"]


def kernel(**inputs) -> np.ndarray:
    from concourse.bass_utils import run_bass_kernel_spmd

    nc = get_nc()
    in_maps = _host_prep(inputs)
    res = run_bass_kernel_spmd(nc, in_maps, core_ids=list(range(NCORES)))
    _CACHE["last_result"] = res
    outs = [res.results[c]["out"] for c in range(NCORES)]
    full = np.stack(
        [np.concatenate([outs[b * 4 + qc] for qc in range(4)], axis=0) for b in range(B)]
    )
    return full.astype(np.float32)



# revision 3
# speedup vs baseline: 1.1854x; 1.1854x over previous
"""Multi-head latent attention (MLA) TRN2 kernel.

Sharding: batch(2) x query-sequence(4) over 8 cores. Each core:
  - computes the full KV path for its batch (kv_a, rmsnorm, kv_b, rope)
  - computes the Q path for its 512-token query chunk
  - full attention for its 512 queries x 2048 keys x 16 heads
  - o_proj for its chunk -> output slice [512, 2048]
Host assembles the 8 slices into [B, T, HID]. No collectives.

All matmul operands are bf16 (1 cycle/row on the PE like f32r, but half
the HBM traffic and no N>=256 constraint); PSUM accumulation and the
softmax statistics (sum-of-squares, rsqrt, denominators, reciprocals)
stay f32. Intermediates (q nope/rope, rotated keys, kv latent, attention
output) never leave SBUF: rope-paired layouts are built with SBUF->SBUF
scatter DMAs. Weights are host-pre-tiled as [128 part, tile, payload] so
every DMA moves >=3KB contiguous runs per partition, and each phase's
weights are prefetched during the previous phase on the ACT DGE queue
while the SP queue carries the current phase's streaming loads.

Activations are kept feature-major ([feature, token]) so weight tiles
act as lhsT directly; attention computes scores transposed
(s^T[k,q] = k^T q) so softmax needs no transposes: exp on ACT, the
denominator via an all-ones lhsT matmul, and P@V consumes the
transposed probabilities directly.
"""

import math

import numpy as np

B, T, HID = 2, 2048, 2048
NH, NKV = 16, 8
NOPE, ROPE = 128, 64
HD = NOPE + ROPE  # 192
VD = 128
KV_RANK, Q_RANK = 512, 1536
EPS = 1e-6
THETA = 10000.0
NCORES = 8
TQ = B * T // NCORES  # 512 query tokens per core
P = 128
SCALE = 1.0 / math.sqrt(HD)

# Rope rows are stored "paired": each head's rotated rope halves (32+32
# rows) are stacked into one contiguous 64-row slot at base partition
# 64*(kvh%2), so the score-matmul lhsT(k)/rhs(q) base partitions match
# (PE only allows bases {0, 32, 64}).

_CACHE = {}


def _build_nc():
    import concourse.bass as bass  # noqa: F401
    import concourse.mybir as mybir
    from concourse import bacc
    from concourse.tile import TileContext

    F32 = mybir.dt.float32
    F32R = mybir.dt.float32r
    BF16 = mybir.dt.bfloat16
    AF = mybir.ActivationFunctionType
    ALU = mybir.AluOpType

    nc = bacc.Bacc(None, target_bir_lowering=False)

    xq_d = nc.dram_tensor("xq", [P, 16, TQ], BF16, kind="ExternalInput")
    xch_d = nc.dram_tensor("xch", [P, 8, 16, 256], BF16, kind="ExternalInput")
    qa_d = nc.dram_tensor("qa_w", [P, 12, 16, P], BF16, kind="ExternalInput")
    qb_d = nc.dram_tensor("qb_w", [P, 24, 12, P], BF16, kind="ExternalInput")
    kva_d = nc.dram_tensor("kva_w", [P, 16, 1024], BF16, kind="ExternalInput")
    kvb_d = nc.dram_tensor("kvb_w", [P, 4, 2048], BF16, kind="ExternalInput")
    o_d = nc.dram_tensor("o_w", [P, 4, 16, 512], BF16, kind="ExternalInput")
    cosq_d = nc.dram_tensor("cosq", [P, TQ], BF16, kind="ExternalInput")
    sinq_d = nc.dram_tensor("sinq", [P, TQ], BF16, kind="ExternalInput")
    cosk_d = nc.dram_tensor("cosk", [P, T], BF16, kind="ExternalInput")
    sink_d = nc.dram_tensor("sink", [P, T], BF16, kind="ExternalInput")
    onesb_d = nc.dram_tensor("ones_b", [P, P], BF16, kind="ExternalInput")
    onesr_d = nc.dram_tensor("ones_r", [P, P], F32R, kind="ExternalInput")
    lnq_d = nc.dram_tensor("lnq", [P, 12], F32, kind="ExternalInput")
    lnkv_d = nc.dram_tensor("lnkv", [P, 4], F32, kind="ExternalInput")
    eps_d = nc.dram_tensor("eps_in", [P, 2], F32, kind="ExternalInput")
    out_d = nc.dram_tensor("out", [TQ, HID], F32, kind="ExternalOutput")

    with TileContext(nc) as tc:
        with tc.tile_pool(name="resident", bufs=1) as res:
            ones_sb = res.tile([P, P], BF16, name="ones_sb")
            nc.sync.dma_start(ones_sb[:], onesb_d[:, :])
            onesr_sb = res.tile([P, P], F32R, name="onesr_sb")
            nc.sync.dma_start(onesr_sb[:], onesr_d[:, :])
            lnq_sb = res.tile([P, 12], F32, name="lnq_sb")
            nc.sync.dma_start(lnq_sb[:], lnq_d[:, :])
            lnkv_sb = res.tile([P, 4], F32, name="lnkv_sb")
            nc.sync.dma_start(lnkv_sb[:], lnkv_d[:, :])
            eps_sb = res.tile([P, 2], F32, name="eps_sb")
            nc.sync.dma_start(eps_sb[:], eps_d[:, :])

            kv_latN = res.tile([P, 4, T], BF16, name="kv_latN")
            qnope = res.tile([P, NH, TQ], BF16, name="qnope")
            qrope = res.tile([P, 8, TQ], BF16, name="qrope")
            kpair = res.tile([P, 4, T], BF16, name="kpair")
            attn_sb = res.tile([P, NH, TQ], BF16, name="attn_sb")
            kvb_sb = res.tile([P, 4, 2048], BF16, name="kvb_sb")

            # -- pf1: P1's inputs, prefetched during P2, freed after P1 ----
            with tc.tile_pool(name="pf1", bufs=1) as pf1:
                kva_sb = pf1.tile([P, 16, 1024], BF16, name="kva_sb")
                cosk_sb = pf1.tile([P, T], BF16, name="cosk_sb")
                sink_sb = pf1.tile([P, T], BF16, name="sink_sb")

                # ------------- P2: q path (first; no kv deps) -------------
                with (
                    tc.tile_pool(name="p2", bufs=1) as p2,
                    tc.tile_pool(name="p2w", bufs=3) as p2w,
                    tc.tile_pool(name="p2s", bufs=2) as p2s,
                    tc.tile_pool(name="p2ps", bufs=2, space="PSUM") as p2ps,
                    tc.tile_pool(name="p2ps1", bufs=1, space="PSUM") as p2ps1,
                ):
                    xq_sb = p2.tile([P, 16, TQ], BF16, name="xq_sb")
                    nc.sync.dma_start(xq_sb[:], xq_d[:, :, :])
                    cosq_sb = p2.tile([P, TQ], BF16, name="cosq_sb")
                    nc.sync.dma_start(cosq_sb[:], cosq_d[:, :])
                    sinq_sb = p2.tile([P, TQ], BF16, name="sinq_sb")
                    nc.sync.dma_start(sinq_sb[:], sinq_d[:, :])
                    q_lat = p2.tile([P, 12, TQ], BF16, name="q_lat")
                    rs_q = p2.tile([P, TQ], F32, name="rs_q")

                    # q_a + rmsnorm
                    sumsq = p2ps1.tile([P, TQ], F32, tag="qsumsq")
                    for m in range(12):
                        wt = p2w.tile([P, 16, P], BF16, tag="w")
                        nc.sync.dma_start(wt[:], qa_d[:, m, :, :])
                        ps = p2ps.tile([P, TQ], F32, tag="mm")
                        for k in range(16):
                            nc.tensor.matmul(
                                ps[:], wt[:, k, :], xq_sb[:, k, :],
                                start=(k == 0), stop=(k == 15),
                            )
                        nc.vector.tensor_copy(q_lat[:, m, :], ps[:])
                        sq = p2s.tile([P, TQ], BF16, tag="sq")
                        nc.scalar.square(sq[:], ps[:])
                        nc.tensor.matmul(
                            sumsq[:], ones_sb[:], sq[:],
                            start=(m == 0), stop=(m == 11),
                        )
                        # prefetch P1's kva weights on the ACT DGE queue
                        if m in (2, 5, 8, 11):
                            c = (m + 1) // 3 - 1
                            nc.scalar.dma_start(
                                kva_sb[:, 4 * c : 4 * c + 4, :],
                                kva_d[:, 4 * c : 4 * c + 4, :],
                            )
                    sqt = p2s.tile([P, TQ], F32, tag="sqt")
                    nc.scalar.activation(sqt[:], sumsq[:], AF.Sqrt, bias=eps_sb[:, 1:2])
                    nc.vector.reciprocal(rs_q[:], sqt[:])
                    for m in range(12):
                        nc.vector.scalar_tensor_tensor(
                            q_lat[:, m, :], q_lat[:, m, :],
                            lnq_sb[:, m : m + 1], rs_q[:],
                            ALU.mult, ALU.mult,
                        )

                    # q_b: nope heads to qnope, rope raw kept for rotation
                    qraw1 = p2.tile([P, 4, TQ], BF16, name="qraw1")
                    qraw2 = p2.tile([P, 4, TQ], BF16, name="qraw2")
                    for m in range(24):
                        wt = p2w.tile([P, 16, P], BF16, tag="w")
                        nc.sync.dma_start(wt[:, :12, :], qb_d[:, m, :, :])
                        ps = p2ps.tile([P, TQ], F32, tag="mm")
                        for k in range(12):
                            nc.tensor.matmul(
                                ps[:], wt[:, k, :], q_lat[:, k, :],
                                start=(k == 0), stop=(k == 11),
                            )
                        if m < 16:
                            nc.scalar.copy(qnope[:, m, :], ps[:])
                        elif m < 20:
                            nc.scalar.copy(qraw1[:, m - 16, :], ps[:])
                        else:
                            nc.scalar.copy(qraw2[:, m - 20, :], ps[:])
                        if m == 4:
                            nc.scalar.dma_start(cosk_sb[:], cosk_d[:, :])
                        elif m == 6:
                            nc.scalar.dma_start(sink_sb[:], sink_d[:, :])

                    # q-rope rotation then scatter to paired SBUF layout
                    cb = cosq_sb[:, None, :].to_broadcast((P, 4, TQ))
                    sb_ = sinq_sb[:, None, :].to_broadcast((P, 4, TQ))
                    qrot1 = p2.tile([P, 4, TQ], BF16, name="qrot1")
                    qrot2 = p2.tile([P, 4, TQ], BF16, name="qrot2")
                    tmp = p2.tile([P, 4, TQ], BF16, name="qrot_tmp1")
                    nc.vector.tensor_tensor(tmp[:], qraw2[:], sb_, ALU.mult)
                    nc.vector.tensor_tensor(qrot1[:], qraw1[:], cb, ALU.mult)
                    nc.vector.tensor_tensor(qrot1[:], qrot1[:], tmp[:], ALU.subtract)
                    tmp2 = p2.tile([P, 4, TQ], BF16, name="qrot_tmp2")
                    nc.vector.tensor_tensor(tmp2[:], qraw1[:], sb_, ALU.mult)
                    nc.vector.tensor_tensor(qrot2[:], qraw2[:], cb, ALU.mult)
                    nc.vector.tensor_tensor(qrot2[:], qrot2[:], tmp2[:], ALU.add)
                    # head qh -> slot 2*(qh//4)+qh%2, base 64*((qh//2)%2)
                    for qh in range(NH):
                        slot = 2 * (qh // 4) + qh % 2
                        bb = 64 * ((qh // 2) % 2)
                        src_r = (qh % 4) * 32
                        nc.sync.dma_start(
                            qrope[bb : bb + 32, slot, :],
                            qrot1[src_r : src_r + 32, qh // 4, :],
                        )
                        nc.sync.dma_start(
                            qrope[bb + 32 : bb + 64, slot, :],
                            qrot2[src_r : src_r + 32, qh // 4, :],
                        )

                # ------------- P1: kv_a + rmsnorm + rope ------------------
                with (
                    tc.tile_pool(name="p1", bufs=1) as p1,
                    tc.tile_pool(name="p1x", bufs=2) as p1x,
                    tc.tile_pool(name="p1s", bufs=2) as p1s,
                    tc.tile_pool(name="p1ps", bufs=2, space="PSUM") as p1ps,
                    tc.tile_pool(name="p1ps1", bufs=1, space="PSUM") as p1ps1,
                ):
                    raw1 = p1.tile([P, 2, T], BF16, name="raw1")
                    raw2 = p1.tile([P, 2, T], BF16, name="raw2")
                    for nch in range(8):
                        chsl = slice(nch * 256, (nch + 1) * 256)
                        xch = p1x.tile([P, 16, 256], BF16, tag="x")
                        nc.sync.dma_start(xch[:], xch_d[:, nch, :, :])
                        sumsq = p1ps1.tile([P, 256], F32, tag="ksumsq")
                        for m in range(8):
                            ps = p1ps.tile([P, 256], F32, tag="mm")
                            for k in range(16):
                                nc.tensor.matmul(
                                    ps[:], kva_sb[:, k, m * P : (m + 1) * P],
                                    xch[:, k, :],
                                    start=(k == 0), stop=(k == 15),
                                )
                            if m < 4:
                                nc.vector.tensor_copy(kv_latN[:, m, chsl], ps[:])
                                sq = p1s.tile([P, 256], BF16, tag="sq")
                                nc.scalar.square(sq[:], ps[:])
                                nc.tensor.matmul(
                                    sumsq[:], ones_sb[:], sq[:],
                                    start=(m == 0), stop=(m == 3),
                                )
                            elif m < 6:
                                nc.scalar.copy(raw1[:, m - 4, chsl], ps[:])
                            else:
                                nc.scalar.copy(raw2[:, m - 6, chsl], ps[:])
                        sqt = p1s.tile([P, 256], F32, tag="sqt")
                        nc.scalar.activation(
                            sqt[:], sumsq[:], AF.Sqrt, bias=eps_sb[:, 0:1]
                        )
                        rs = p1s.tile([P, 256], F32, tag="rs")
                        nc.vector.reciprocal(rs[:], sqt[:])
                        for m in range(4):
                            nc.vector.scalar_tensor_tensor(
                                kv_latN[:, m, chsl], kv_latN[:, m, chsl],
                                lnkv_sb[:, m : m + 1], rs[:],
                                ALU.mult, ALU.mult,
                            )
                        if nch == 1:
                            # prefetch P3's kvb weights on the ACT DGE queue
                            nc.scalar.dma_start(kvb_sb[:], kvb_d[:, :, :])
                        if nch % 2 == 1:
                            # rotate the finished 512-token slab and scatter
                            sl2 = slice((nch - 1) * 256, (nch + 1) * 256)
                            ckb = cosk_sb[:, None, sl2].to_broadcast((P, 2, 512))
                            skb = sink_sb[:, None, sl2].to_broadcast((P, 2, 512))
                            rt = p1s.tile([P, 2, 512], BF16, tag="rtmp")
                            r1 = p1s.tile([P, 2, 512], BF16, tag="rot1")
                            nc.vector.tensor_tensor(rt[:], raw2[:, :, sl2], skb, ALU.mult)
                            nc.vector.tensor_tensor(r1[:], raw1[:, :, sl2], ckb, ALU.mult)
                            nc.vector.tensor_tensor(r1[:], r1[:], rt[:], ALU.subtract)
                            rt2 = p1s.tile([P, 2, 512], BF16, tag="rtmp")
                            r2 = p1s.tile([P, 2, 512], BF16, tag="rot2")
                            nc.vector.tensor_tensor(rt2[:], raw1[:, :, sl2], skb, ALU.mult)
                            nc.vector.tensor_tensor(r2[:], raw2[:, :, sl2], ckb, ALU.mult)
                            nc.vector.tensor_tensor(r2[:], r2[:], rt2[:], ALU.add)
                            # head kvh -> slot kvh//2, base 64*(kvh%2)
                            for kvh in range(NKV):
                                t_, i = kvh // 4, kvh % 4
                                bb = 64 * (kvh % 2)
                                nc.sync.dma_start(
                                    kpair[bb : bb + 32, kvh // 2, sl2],
                                    r1[i * 32 : (i + 1) * 32, t_, :],
                                )
                                nc.sync.dma_start(
                                    kpair[bb + 32 : bb + 64, kvh // 2, sl2],
                                    r2[i * 32 : (i + 1) * 32, t_, :],
                                )

            # ------------- P3 + P4 (pf1 SBUF freed) -----------------------
            with tc.tile_pool(name="oww", bufs=2) as oww:
                ow_tiles = {}

                def ow_load(n, eng):
                    ow = oww.tile([P, 16, 512], BF16, tag="ow")
                    eng.dma_start(ow[:], o_d[:, n, :, :])
                    ow_tiles[n] = ow

                with (
                    tc.tile_pool(name="p3", bufs=2) as p3,
                    tc.tile_pool(name="p3q", bufs=4) as p3q,
                    tc.tile_pool(name="p3p", bufs=3) as p3p,
                    tc.tile_pool(name="scps", bufs=3, space="PSUM") as scps,
                    tc.tile_pool(name="atps", bufs=2, space="PSUM") as atps,
                    tc.tile_pool(name="prps", bufs=2, space="PSUM") as prps,
                ):
                    pending = []

                    def finalize(item):
                        dsum, at, qh = item
                        dn = scps.tile([P, TQ], F32, tag="sc")
                        nc.tensor.matmul(
                            dn[:], onesr_sb[:], dsum[:], start=True, stop=True
                        )
                        rec = p3q.tile([P, TQ], F32, tag="rec")
                        nc.vector.reciprocal(rec[:], dn[:])
                        nc.vector.tensor_tensor(
                            attn_sb[:, qh, :], at[:], rec[:], ALU.mult
                        )

                    for hp in range(4):  # kv-head pairs
                        kvh0 = 2 * hp
                        knp = p3.tile([P, 2, T], BF16, tag="knp")
                        for h2 in range(2):
                            wsl = slice((kvh0 + h2) * NOPE, (kvh0 + h2 + 1) * NOPE)
                            for n4 in range(4):
                                ksl = slice(n4 * 512, (n4 + 1) * 512)
                                ps = prps.tile([P, 512], F32, tag="pre")
                                for r in range(4):
                                    nc.tensor.matmul(
                                        ps[:], kvb_sb[:, r, wsl],
                                        kv_latN[:, r, ksl],
                                        start=(r == 0), stop=(r == 3),
                                    )
                                nc.vector.tensor_copy(knp[:, h2, ksl], ps[:])
                        vp = p3.tile([P, 16, 256], BF16, tag="vp")
                        vsl = slice(NKV * NOPE + kvh0 * VD, NKV * NOPE + (kvh0 + 2) * VD)
                        for kt in range(16):
                            ps = prps.tile([P, 512], F32, tag="pre")
                            for r in range(4):
                                nc.tensor.matmul(
                                    ps[:, :256],
                                    kv_latN[:, r, kt * P : (kt + 1) * P],
                                    kvb_sb[:, r, vsl],
                                    start=(r == 0), stop=(r == 3),
                                )
                            nc.scalar.copy(vp[:, kt, :], ps[:, :256])

                        for j4 in range(4):
                            qh = 4 * hp + j4
                            kvh = qh // 2
                            h2 = kvh - kvh0
                            b = 64 * (kvh % 2)
                            slot = 2 * (qh // 4) + qh % 2
                            dsum = p3q.tile([P, TQ], F32R, tag="dsum")
                            at = atps.tile([P, TQ], F32, tag="at")
                            pts = {}
                            for kt in range(16):
                                sc = scps.tile([P, TQ], F32, tag="sc")
                                nc.tensor.matmul(
                                    sc[:],
                                    knp[:, h2, kt * P : (kt + 1) * P],
                                    qnope[:, qh, :],
                                    start=True, stop=False,
                                )
                                nc.tensor.matmul(
                                    sc[:],
                                    kpair[b : b + 64, kvh // 2, kt * P : (kt + 1) * P],
                                    qrope[b : b + 64, slot, :],
                                    start=False, stop=True,
                                )
                                pt = p3p.tile([P, TQ], BF16, tag="pt")
                                nc.scalar.activation(
                                    pt[:], sc[:], AF.Exp, scale=float(SCALE)
                                )
                                pts[kt] = pt
                                if kt == 0:
                                    nc.vector.tensor_copy(dsum[:], pt[:])
                                else:
                                    nc.vector.tensor_tensor(
                                        dsum[:], dsum[:], pt[:], ALU.add
                                    )
                                if kt > 0:  # PV one stage behind scores
                                    nc.tensor.matmul(
                                        at[:],
                                        vp[:, kt - 1, h2 * VD : (h2 + 1) * VD],
                                        pts[kt - 1][:],
                                        start=(kt == 1), stop=False,
                                    )
                                    del pts[kt - 1]
                            nc.tensor.matmul(
                                at[:],
                                vp[:, 15, h2 * VD : (h2 + 1) * VD],
                                pts[15][:],
                                start=False, stop=True,
                            )
                            pending.append((dsum, at, qh))
                            if len(pending) == 2:
                                finalize(pending.pop(0))
                        # prefetch P4's o_proj weights on the ACT DGE queue
                        if hp == 2:
                            ow_load(0, nc.scalar)
                        elif hp == 3:
                            ow_load(1, nc.scalar)
                    while pending:
                        finalize(pending.pop(0))

                # ------------- P4: o_proj (attn_sb resident) --------------
                with (
                    tc.tile_pool(name="p4s", bufs=2) as p4s,
                    tc.tile_pool(name="p4ps", bufs=2, space="PSUM") as p4ps,
                ):
                    for n in range(4):
                        if n not in ow_tiles:
                            ow_load(n, nc.sync)
                        ow = ow_tiles[n]
                        for mt in range(4):
                            ps = p4ps.tile([P, 512], F32, tag="o")
                            for h in range(NH):
                                nc.tensor.matmul(
                                    ps[:],
                                    attn_sb[:, h, mt * P : (mt + 1) * P],
                                    ow[:, h, :],
                                    start=(h == 0), stop=(h == 15),
                                )
                            st = p4s.tile([P, 512], F32, tag="st")
                            nc.scalar.copy(st[:], ps[:])
                            nc.sync.dma_start(
                                out_d[mt * P : (mt + 1) * P, n * 512 : (n + 1) * 512],
                                st[:],
                            )

    nc.finalize()
    return nc


def _host_prep(inputs):
    import ml_dtypes

    BF = ml_dtypes.bfloat16

    def bf(a):
        return np.ascontiguousarray(np.asarray(a, dtype=np.float32).astype(BF))

    x = np.asarray(inputs["hidden_states"], dtype=np.float32)

    qa_w = np.asarray(inputs["q_a_w"], np.float32)  # [HID, Q_RANK]
    qa_t = bf(qa_w.reshape(16, P, 12, P).transpose(1, 2, 0, 3))

    qb = np.asarray(inputs["q_b_w"], np.float32).reshape(Q_RANK, NH, HD)
    nope_cols = qb[:, :, :NOPE].reshape(Q_RANK, NH * NOPE)
    rope1 = qb[:, :, NOPE : NOPE + 32].reshape(Q_RANK, NH * 32)
    rope2 = qb[:, :, NOPE + 32 :].reshape(Q_RANK, NH * 32)
    qb_cols = np.concatenate([nope_cols, rope1, rope2], axis=1)  # [1536, 3072]
    qb_t = bf(qb_cols.reshape(12, P, 24, P).transpose(1, 2, 0, 3))

    kva = np.asarray(inputs["kv_a_w"], np.float32)
    lat = kva[:, :KV_RANK]
    krope = kva[:, KV_RANK:].reshape(HID, NKV, ROPE)
    kr1 = krope[:, :, :32].reshape(HID, NKV * 32)
    kr2 = krope[:, :, 32:].reshape(HID, NKV * 32)
    kva_cols = np.concatenate([lat, kr1, kr2], axis=1)  # [2048, 1024]
    kva_t = bf(kva_cols.reshape(16, P, 1024).transpose(1, 0, 2))

    kvb = np.asarray(inputs["kv_b_w"], np.float32).reshape(KV_RANK, NKV, NOPE + VD)
    knope_cols = kvb[:, :, :NOPE].reshape(KV_RANK, NKV * NOPE)
    v_cols = kvb[:, :, NOPE:].reshape(KV_RANK, NKV * VD)
    kvb_cols = np.concatenate([knope_cols, v_cols], axis=1)  # [512, 2048]
    kvb_t = bf(kvb_cols.reshape(4, P, 2048).transpose(1, 0, 2))

    o_w = np.asarray(inputs["o_w"], np.float32)  # [NH*VD, HID]
    o_t = bf(o_w.reshape(16, P, 4, 512).transpose(1, 2, 0, 3))

    lnq = (
        (np.asarray(inputs["q_a_ln_w"], np.float64) * math.sqrt(Q_RANK))
        .astype(np.float32)
        .reshape(12, P)
        .T.copy()
    )
    lnkv = (
        (np.asarray(inputs["kv_a_ln_w"], np.float64) * math.sqrt(KV_RANK))
        .astype(np.float32)
        .reshape(4, P)
        .T.copy()
    )

    inv_freq = 1.0 / (THETA ** (np.arange(0, ROPE, 2, dtype=np.float32) / ROPE))
    t = np.arange(T, dtype=np.float32)
    freqs = np.outer(t, inv_freq).astype(np.float32)
    cosk = np.tile(np.cos(freqs).T, (4, 1))  # [128, T]
    sink = np.tile(np.sin(freqs).T, (4, 1))
    cosk_b, sink_b = bf(cosk), bf(sink)
    ones_b = np.ones((P, P), BF)
    ones_r = np.ones((P, P), np.float32)
    eps2 = np.empty((P, 2), np.float32)
    eps2[:, 0] = EPS * KV_RANK
    eps2[:, 1] = EPS * Q_RANK

    in_maps = []
    for c in range(NCORES):
        b, qc = c // 4, c % 4
        xTb = x[b].T  # [HID, T]
        qoff = qc * TQ
        xch_t = bf(xTb.reshape(16, P, 8, 256).transpose(1, 2, 0, 3))
        xq_t = bf(xTb[:, qoff : qoff + TQ].reshape(16, P, TQ).transpose(1, 0, 2))
        in_maps.append(
            {
                "xq": xq_t,
                "xch": xch_t,
                "qa_w": qa_t,
                "qb_w": qb_t,
                "kva_w": kva_t,
                "kvb_w": kvb_t,
                "o_w": o_t,
                "cosq": np.ascontiguousarray(cosk_b[:, qoff : qoff + TQ]),
                "sinq": np.ascontiguousarray(sink_b[:, qoff : qoff + TQ]),
                "cosk": cosk_b,
                "sink": sink_b,
                "ones_b": ones_b,
                "ones_r": ones_r,
                "lnq": lnq,
                "lnkv": lnkv,
                "eps_in": eps2,
            }
        )
    return in_maps


def get_nc():
    if "nc" not in _CACHE:
        _CACHE["nc"] = _build_nc()
    return _CACHE["nc"]


def kernel(**inputs) -> np.ndarray:
    from concourse.bass_utils import run_bass_kernel_spmd

    nc = get_nc()
    in_maps = _host_prep(inputs)
    res = run_bass_kernel_spmd(nc, in_maps, core_ids=list(range(NCORES)))
    _CACHE["last_result"] = res
    outs = [res.results[c]["out"] for c in range(NCORES)]
    full = np.stack(
        [np.concatenate([outs[b * 4 + qc] for qc in range(4)], axis=0) for b in range(B)]
    )
    return full.astype(np.float32)
